# revision 15
# baseline (speedup 1.0000x reference)
"""Trainium2 Bass kernel for CompositionalPhoneticsModel (segment_reduce).

Computation (reference):
    phone   = einsum('bth,hp->btp', enc_output, feature2phone) / sqrt(H)
    allo    = where(mapping>0, phone[:,:,None,:]*mapping, -inf)   # mapping is 0/1
    phoneme = max(allo, axis=-1)                                  # masked segment max
    out     = log_softmax(phoneme, axis=2)

Device strategy (8 NeuronCores, data-parallel over the B*T=8192 rows):
  * Host gathers feature2phone columns into segment-contiguous order
    (phones in 2 segments get duplicated columns; NNZ ~ 460), folds in the
    1/sqrt(H) scale, and sorts segments by length so the per-segment max is a
    handful of strided DVE reduce_max ops.  The device phoneme order is a
    permutation of 0..95; max/logsumexp are permutation-invariant so the host
    un-permutes the output columns at the end.
  * Host pre-casts enc to bf16 and pre-interleaves it as [128, rows, NH]
    (partition p holds h = c*128+p for all 5 contraction chunks c) so each
    DMA moves multi-KB contiguous lines per partition and the matmul lhsT
    tiles are direct slices — no on-chip transposes.
  * log-softmax without max-subtraction (phone logits are ~N(0,1); exp fits
    fp32 comfortably): exp on ScalarE (one table load), row-sums and the
    final x - ln(sum) on VectorE.
"""

from contextlib import ExitStack

import numpy as np
import ml_dtypes

import concourse.bass as bass
import concourse.bacc as bacc
import concourse.tile as tile
from concourse import mybir
from concourse.bass_utils import run_bass_kernel_spmd

B, T, H = 8, 1024, 640
N_PHONEME, N_PHONE = 96, 230
N_CORES = 8
ROWS = B * T
RC = ROWS // N_CORES          # rows per core
NH = H // 128                 # contraction chunks
TW = 512                      # rows per enc DMA megatile
NB = RC // 128                # 128-row blocks per core
BF16 = ml_dtypes.bfloat16


def _structure(mapping: np.ndarray):
    """Segment-contiguous gather order, grouped by segment length (desc).

    Returns (col_ids, groups, perm):
      col_ids: phone index feeding each device matmul column (len NNZ)
      groups:  list of (L, nL, col_off, out_off) — nL segments of length L
               occupy matmul cols [col_off, col_off+nL*L) and device output
               cols [out_off, out_off+nL)
      perm:    perm[j] = original phoneme id of device output column j
    """
    segs = [np.nonzero(mapping[m] > 0)[0] for m in range(N_PHONEME)]
    lengths = np.array([len(s) for s in segs])
    assert lengths.min() >= 1
    order = np.argsort(-lengths, kind="stable")
    col_ids, groups, perm = [], [], []
    i = 0
    while i < N_PHONEME:
        L = int(lengths[order[i]])
        j = i
        while j < N_PHONEME and lengths[order[j]] == L:
            j += 1
        groups.append((L, j - i, len(col_ids), i))
        for k in range(i, j):
            m = int(order[k])
            col_ids.extend(segs[m].tolist())
            perm.append(m)
        i = j
    return np.array(col_ids, dtype=np.int64), groups, np.array(perm, dtype=np.int64)


def _patch_act_tables():
    """Make Exp and Ln resolve to the same activation-table set.

    bacc's insert_act_table_loads models a single table slot, so a kernel
    alternating Exp/Ln reloads a 1.3us table on every transition.  act_info
    has a joint set ('natural_log_exp_and_others') containing both; keep the
    set list's order/indices intact but strip Exp/Ln from the other sets so
    the pass picks the joint set for both and emits a single load.
    """
    if getattr(bacc, "_act_tables_patched", False):
        return
    from concourse import hw_specs
    orig = hw_specs.get_activation_tables
    act = mybir.ActivationFunctionType

    def patched(module_arch):
        tabs = orig(module_arch)
        joint = [k for k, v in tabs.items() if act.Exp in v and act.Ln in v]
        if not joint:
            return tabs
        j = joint[0]
        return {
            k: (v if k == j else (v - {act.Exp, act.Ln}))
            for k, v in tabs.items()
        }

    bacc.get_activation_tables = patched
    bacc._act_tables_patched = True


def _build_program(nnz: int, groups):
    """Build + compile the per-core Bass program. Returns the Bacc object."""
    _patch_act_tables()
    nc = bacc.Bacc("TRN2", target_bir_lowering=False, debug=False)
    dt = mybir.dt
    act = mybir.ActivationFunctionType

    # enc interleaved: [128, RC, NH]; element (p, r, c) = enc[r, c*128+p]
    enck_d = nc.dram_tensor("enck", [128, RC, NH], dt.bfloat16, kind="ExternalInput")
    # W interleaved: [128, NH, nnz]; element (p, c, n) = W[c*128+p, n]
    wk_d = nc.dram_tensor("wk", [128, NH, nnz], dt.bfloat16, kind="ExternalInput")
    # out packed: [128, NB, 96]; element (p, b, m) = out[b*128+p, m]
    out_d = nc.dram_tensor("out", [128, NB, N_PHONEME], dt.float32, kind="ExternalOutput")

    RT = TW // 128
    NMT = RC // TW

    with ExitStack() as ctx:
        tc = ctx.enter_context(tile.TileContext(nc))
        wpool = ctx.enter_context(tc.tile_pool(name="wpool", bufs=1))
        epool = ctx.enter_context(tc.tile_pool(name="epool", bufs=2))
        ppool = ctx.enter_context(tc.tile_pool(name="ppool", bufs=2, space="PSUM"))
        spool = ctx.enter_context(tc.tile_pool(name="spool", bufs=2))

        # W chunk 0 lands first so the c=0 matmuls can start sooner
        wt = wpool.tile([128, NH, nnz], dt.bfloat16)
        nc.sync.dma_start(wt[:, :1, :], wk_d[:, :1, :])
        nc.sync.dma_start(wt[:, 1:, :], wk_d[:, 1:, :])

        # PE warmup: dummy matmuls on zeroed scratch run while the first
        # DMAs land, ramping the tensor engine's p-state so the real
        # stream starts at full speed.  They write the first megatile's
        # PSUM bank; the real accumulation (start=True) overwrites it.
        # memset on the otherwise-idle GpSimd so the warmups start early.
        wu = wpool.tile([128, 512], dt.bfloat16)
        nc.gpsimd.memset(wu[:], 0.0)
        ps0 = ppool.tile([128, RT, 512], dt.float32, tag="ps")
        for _ in range(8):
            nc.tensor.matmul(ps0[:, 0, :], wu[:, :128], wu[:], start=True, stop=True)

        for mt in range(NMT):
            et = epool.tile([128, TW, NH], dt.bfloat16)
            # issue enc loads from the Activation HWDGE so they don't
            # serialize behind the Sync engine's descriptor generation
            nc.scalar.dma_start(et[:], enck_d[:, mt * TW:(mt + 1) * TW, :])

            # one PSUM bank per block
            ps = ps0 if mt == 0 else ppool.tile([128, RT, 512], dt.float32, tag="ps")
            for r in range(RT):
                for c in range(NH):
                    nc.tensor.matmul(
                        ps[:, r, :nnz],
                        et[:, r * 128:(r + 1) * 128, c],
                        wt[:, c, :],
                        start=(c == 0),
                        stop=(c == NH - 1),
                    )

            # segment max: one strided reduce per length group, batched over
            # all RT row blocks (4D input AP [128, RT, nL, L])
            pmax = spool.tile([128, RT, N_PHONEME], dt.float32, tag="pmax")
            for (L, nL, coff, ooff) in groups:
                src = ps[:, :, coff:coff + nL * L].rearrange(
                    "p r (s l) -> p r s l", l=L
                )
                nc.vector.reduce_max(
                    pmax[:, :, ooff:ooff + nL], src, axis=mybir.AxisListType.X
                )

            # exp (no max-subtraction needed: |phone| <~ 7, exp fits fp32)
            ex = spool.tile([128, RT, N_PHONEME], dt.float32, tag="ex")
            for r in range(RT):
                nc.scalar.activation(ex[:, r, :], pmax[:, r, :], act.Exp)
            se = spool.tile([128, RT], dt.float32, tag="se")
            nc.vector.reduce_sum(se[:], ex[:], axis=mybir.AxisListType.X)
            lse = spool.tile([128, RT], dt.float32, tag="lse")
            nc.scalar.activation(lse[:], se[:], act.Ln)
            ott = spool.tile([128, RT, N_PHONEME], dt.float32, tag="ott")
            for r in range(RT):
                nc.vector.tensor_scalar_sub(
                    ott[:, r, :], pmax[:, r, :], lse[:, r:r + 1]
                )
            nc.sync.dma_start(out_d[:, mt * RT:(mt + 1) * RT, :], ott[:])

    nc.compile()
    return nc


_CACHE: dict = {}


def _get_compiled(mapping: np.ndarray):
    key = mapping.astype(np.float32).tobytes()
    if _CACHE.get("key") != key:
        col_ids, groups, perm = _structure(mapping)
        nc = _build_program(len(col_ids), groups)
        _CACHE.update(key=key, col_ids=col_ids, groups=groups, perm=perm, nc=nc)
    return _CACHE["nc"], _CACHE["col_ids"], _CACHE["perm"]


def _prep_in_maps(enc_output, feature2phone, col_ids):
    scale = np.float32(1.0) / np.sqrt(np.float32(H))
    wg = (feature2phone.astype(np.float32) * scale)[:, col_ids].astype(BF16)
    # [H, nnz] -> [128, NH, nnz]
    wk = np.ascontiguousarray(wg.reshape(NH, 128, -1).transpose(1, 0, 2))
    # enc [ROWS, H] -> [128, ROWS, NH]
    e3 = enc_output.astype(BF16).reshape(ROWS, NH, 128)
    enck = np.ascontiguousarray(e3.transpose(2, 0, 1))
    in_maps = []
    for c in range(N_CORES):
        in_maps.append({
            "enck": np.ascontiguousarray(enck[:, c * RC:(c + 1) * RC, :]),
            "wk": wk,
        })
    return in_maps


def run_device(enc_output, feature2phone, mapping, trace=False, **kw):
    """Build/compile (cached), run on the 8 cores, return (output, BassKernelResults)."""
    enc_output = np.asarray(enc_output)
    feature2phone = np.asarray(feature2phone)
    mapping = np.asarray(mapping)
    nc, col_ids, perm = _get_compiled(mapping)
    in_maps = _prep_in_maps(enc_output, feature2phone, col_ids)
    res = run_bass_kernel_spmd(
        nc, in_maps, core_ids=list(range(N_CORES)), trace=trace, **kw
    )
    # device out [128, NB, 96] packed -> rows b*128+p
    dev = np.concatenate(
        [res.results[c]["out"].transpose(1, 0, 2).reshape(RC, N_PHONEME)
         for c in range(N_CORES)],
        axis=0,
    )
    out = np.empty_like(dev)
    out[:, perm] = dev
    return out.reshape(B, T, N_PHONEME).astype(np.float32), res


def kernel(enc_output, feature2phone, mapping):
    out, _ = run_device(enc_output, feature2phone, mapping)
    return out


# revision 18
# speedup vs baseline: 1.0915x; 1.0915x over previous
"""Trainium2 Bass kernel for CompositionalPhoneticsModel (segment_reduce).

Computation (reference):
    phone   = einsum('bth,hp->btp', enc_output, feature2phone) / sqrt(H)
    allo    = where(mapping>0, phone[:,:,None,:]*mapping, -inf)   # mapping is 0/1
    phoneme = max(allo, axis=-1)                                  # masked segment max
    out     = log_softmax(phoneme, axis=2)

Device strategy (8 NeuronCores, data-parallel over the B*T=8192 rows):
  * Host gathers feature2phone columns into segment-contiguous order
    (phones in 2 segments get duplicated columns; NNZ ~ 460), folds in the
    1/sqrt(H) scale, and sorts segments by length so the per-segment max is a
    handful of strided DVE reduce_max ops.  The device phoneme order is a
    permutation of 0..95; max/logsumexp are permutation-invariant so the host
    un-permutes the output columns at the end.
  * Host pre-casts enc to bf16 and pre-interleaves it as [128, rows, NH]
    (partition p holds h = c*128+p for all 5 contraction chunks c) so each
    DMA moves multi-KB contiguous lines per partition and the matmul lhsT
    tiles are direct slices — no on-chip transposes.
  * log-softmax without max-subtraction (phone logits are ~N(0,1); exp fits
    fp32 comfortably): exp on ScalarE (one table load), row-sums and the
    final x - ln(sum) on VectorE.
"""

from contextlib import ExitStack

import numpy as np
import ml_dtypes

import concourse.bass as bass
import concourse.bacc as bacc
import concourse.tile as tile
from concourse import mybir
from concourse.bass_utils import run_bass_kernel_spmd

B, T, H = 8, 1024, 640
N_PHONEME, N_PHONE = 96, 230
N_CORES = 8
ROWS = B * T
RC = ROWS // N_CORES          # rows per core
NH = H // 128                 # contraction chunks
TW = 256                      # rows per enc DMA megatile
NB = RC // 128                # 128-row blocks per core
BF16 = ml_dtypes.bfloat16


def _structure(mapping: np.ndarray):
    """Segment-contiguous gather order, grouped by segment length (desc).

    Returns (col_ids, groups, perm):
      col_ids: phone index feeding each device matmul column (len NNZ)
      groups:  list of (L, nL, col_off, out_off) — nL segments of length L
               occupy matmul cols [col_off, col_off+nL*L) and device output
               cols [out_off, out_off+nL)
      perm:    perm[j] = original phoneme id of device output column j
    """
    segs = [np.nonzero(mapping[m] > 0)[0] for m in range(N_PHONEME)]
    lengths = np.array([len(s) for s in segs])
    assert lengths.min() >= 1
    order = np.argsort(-lengths, kind="stable")
    col_ids, groups, perm = [], [], []
    i = 0
    while i < N_PHONEME:
        L = int(lengths[order[i]])
        j = i
        while j < N_PHONEME and lengths[order[j]] == L:
            j += 1
        groups.append((L, j - i, len(col_ids), i))
        for k in range(i, j):
            m = int(order[k])
            col_ids.extend(segs[m].tolist())
            perm.append(m)
        i = j
    return np.array(col_ids, dtype=np.int64), groups, np.array(perm, dtype=np.int64)


def _patch_act_tables():
    """Make Exp and Ln resolve to the same activation-table set.

    bacc's insert_act_table_loads models a single table slot, so a kernel
    alternating Exp/Ln reloads a 1.3us table on every transition.  act_info
    has a joint set ('natural_log_exp_and_others') containing both; keep the
    set list's order/indices intact but strip Exp/Ln from the other sets so
    the pass picks the joint set for both and emits a single load.
    """
    if getattr(bacc, "_act_tables_patched", False):
        return
    from concourse import hw_specs
    orig = hw_specs.get_activation_tables
    act = mybir.ActivationFunctionType

    def patched(module_arch):
        tabs = orig(module_arch)
        joint = [k for k, v in tabs.items() if act.Exp in v and act.Ln in v]
        if not joint:
            return tabs
        j = joint[0]
        return {
            k: (v if k == j else (v - {act.Exp, act.Ln}))
            for k, v in tabs.items()
        }

    bacc.get_activation_tables = patched
    bacc._act_tables_patched = True


def _build_program(nnz: int, groups):
    """Build + compile the per-core Bass program. Returns the Bacc object."""
    _patch_act_tables()
    nc = bacc.Bacc("TRN2", target_bir_lowering=False, debug=False)
    dt = mybir.dt
    act = mybir.ActivationFunctionType

    # enc interleaved: [128, RC, NH]; element (p, r, c) = enc[r, c*128+p]
    enck_d = nc.dram_tensor("enck", [128, RC, NH], dt.bfloat16, kind="ExternalInput")
    # W interleaved: [128, NH, nnz]; element (p, c, n) = W[c*128+p, n]
    wk_d = nc.dram_tensor("wk", [128, NH, nnz], dt.bfloat16, kind="ExternalInput")
    # out packed: [128, NB, 96]; element (p, b, m) = out[b*128+p, m]
    out_d = nc.dram_tensor("out", [128, NB, N_PHONEME], dt.float32, kind="ExternalOutput")

    RT = TW // 128
    NMT = RC // TW

    with ExitStack() as ctx:
        tc = ctx.enter_context(tile.TileContext(nc))
        wpool = ctx.enter_context(tc.tile_pool(name="wpool", bufs=1))
        epool = ctx.enter_context(tc.tile_pool(name="epool", bufs=3))
        ppool = ctx.enter_context(tc.tile_pool(name="ppool", bufs=4, space="PSUM"))
        spool = ctx.enter_context(tc.tile_pool(name="spool", bufs=2))

        # W chunk 0 lands first so the c=0 matmuls can start sooner
        wt = wpool.tile([128, NH, nnz], dt.bfloat16)
        nc.sync.dma_start(wt[:, :1, :], wk_d[:, :1, :])
        nc.sync.dma_start(wt[:, 1:, :], wk_d[:, 1:, :])

        # PE warmup: dummy matmuls on zeroed scratch run while the first
        # DMAs land, ramping the tensor engine's p-state so the real
        # stream starts at full speed.  They write the first megatile's
        # PSUM bank; the real accumulation (start=True) overwrites it.
        # memset on the otherwise-idle GpSimd so the warmups start early
        wu = wpool.tile([128, 512], dt.bfloat16)
        nc.gpsimd.memset(wu[:], 0.0)
        ps0 = ppool.tile([128, RT, 512], dt.float32, tag="ps")
        for _ in range(8):
            nc.tensor.matmul(ps0[:, 0, :], wu[:, :128], wu[:], start=True, stop=True)

        for mt in range(NMT):
            et = epool.tile([128, TW, NH], dt.bfloat16)
            # issue enc loads from the Activation HWDGE so they don't
            # serialize behind the Sync engine's descriptor generation
            nc.scalar.dma_start(et[:], enck_d[:, mt * TW:(mt + 1) * TW, :])

            # one PSUM bank per block
            ps = ps0 if mt == 0 else ppool.tile([128, RT, 512], dt.float32, tag="ps")
            for r in range(RT):
                for c in range(NH):
                    nc.tensor.matmul(
                        ps[:, r, :nnz],
                        et[:, r * 128:(r + 1) * 128, c],
                        wt[:, c, :],
                        start=(c == 0),
                        stop=(c == NH - 1),
                    )

            # segment max: one strided reduce per length group, batched over
            # all RT row blocks (4D input AP [128, RT, nL, L])
            pmax = spool.tile([128, RT, N_PHONEME], dt.float32, tag="pmax")
            for (L, nL, coff, ooff) in groups:
                src = ps[:, :, coff:coff + nL * L].rearrange(
                    "p r (s l) -> p r s l", l=L
                )
                nc.vector.reduce_max(
                    pmax[:, :, ooff:ooff + nL], src, axis=mybir.AxisListType.X
                )

            # exp (no max-subtraction needed: |phone| <~ 7, exp fits fp32);
            # row-sum comes free via the activation accumulator
            ex = spool.tile([128, RT, N_PHONEME], dt.float32, tag="ex")
            se = spool.tile([128, RT], dt.float32, tag="se")
            for r in range(RT):
                nc.scalar.activation(ex[:, r, :], pmax[:, r, :], act.Exp,
                                     accum_out=se[:, r:r + 1])
            lse = spool.tile([128, RT], dt.float32, tag="lse")
            nc.scalar.activation(lse[:], se[:], act.Ln)
            ott = spool.tile([128, RT, N_PHONEME], dt.float32, tag="ott")
            for r in range(RT):
                nc.vector.tensor_scalar_sub(
                    ott[:, r, :], pmax[:, r, :], lse[:, r:r + 1]
                )
            nc.sync.dma_start(out_d[:, mt * RT:(mt + 1) * RT, :], ott[:])

    nc.compile()
    return nc


_CACHE: dict = {}


def _get_compiled(mapping: np.ndarray):
    key = mapping.astype(np.float32).tobytes()
    if _CACHE.get("key") != key:
        col_ids, groups, perm = _structure(mapping)
        nc = _build_program(len(col_ids), groups)
        _CACHE.update(key=key, col_ids=col_ids, groups=groups, perm=perm, nc=nc)
    return _CACHE["nc"], _CACHE["col_ids"], _CACHE["perm"]


def _prep_in_maps(enc_output, feature2phone, col_ids):
    scale = np.float32(1.0) / np.sqrt(np.float32(H))
    wg = (feature2phone.astype(np.float32) * scale)[:, col_ids].astype(BF16)
    # [H, nnz] -> [128, NH, nnz]
    wk = np.ascontiguousarray(wg.reshape(NH, 128, -1).transpose(1, 0, 2))
    # enc [ROWS, H] -> [128, ROWS, NH]
    e3 = enc_output.astype(BF16).reshape(ROWS, NH, 128)
    enck = np.ascontiguousarray(e3.transpose(2, 0, 1))
    in_maps = []
    for c in range(N_CORES):
        in_maps.append({
            "enck": np.ascontiguousarray(enck[:, c * RC:(c + 1) * RC, :]),
            "wk": wk,
        })
    return in_maps


def run_device(enc_output, feature2phone, mapping, trace=False, **kw):
    """Build/compile (cached), run on the 8 cores, return (output, BassKernelResults)."""
    enc_output = np.asarray(enc_output)
    feature2phone = np.asarray(feature2phone)
    mapping = np.asarray(mapping)
    nc, col_ids, perm = _get_compiled(mapping)
    in_maps = _prep_in_maps(enc_output, feature2phone, col_ids)
    res = run_bass_kernel_spmd(
        nc, in_maps, core_ids=list(range(N_CORES)), trace=trace, **kw
    )
    # device out [128, NB, 96] packed -> rows b*128+p
    dev = np.concatenate(
        [res.results[c]["out"].transpose(1, 0, 2).reshape(RC, N_PHONEME)
         for c in range(N_CORES)],
        axis=0,
    )
    out = np.empty_like(dev)
    out[:, perm] = dev
    return out.reshape(B, T, N_PHONEME).astype(np.float32), res


def kernel(enc_output, feature2phone, mapping):
    out, _ = run_device(enc_output, feature2phone, mapping)
    return out


# revision 19
# speedup vs baseline: 1.1438x; 1.0479x over previous
"""Trainium2 Bass kernel for CompositionalPhoneticsModel (segment_reduce).

Computation (reference):
    phone   = einsum('bth,hp->btp', enc_output, feature2phone) / sqrt(H)
    allo    = where(mapping>0, phone[:,:,None,:]*mapping, -inf)   # mapping is 0/1
    phoneme = max(allo, axis=-1)                                  # masked segment max
    out     = log_softmax(phoneme, axis=2)

Device strategy (8 NeuronCores, data-parallel over the B*T=8192 rows):
  * Host gathers feature2phone columns into segment-contiguous order
    (phones in 2 segments get duplicated columns; NNZ ~ 460), folds in the
    1/sqrt(H) scale, and sorts segments by length so the per-segment max is a
    handful of strided DVE reduce_max ops.  The device phoneme order is a
    permutation of 0..95; max/logsumexp are permutation-invariant so the host
    un-permutes the output columns at the end.
  * Host pre-casts enc to bf16 and pre-interleaves it as [128, rows, NH]
    (partition p holds h = c*128+p for all 5 contraction chunks c) so each
    DMA moves multi-KB contiguous lines per partition and the matmul lhsT
    tiles are direct slices — no on-chip transposes.
  * log-softmax without max-subtraction (phone logits are ~N(0,1); exp fits
    fp32 comfortably): exp on ScalarE (one table load), row-sums and the
    final x - ln(sum) on VectorE.
"""

from contextlib import ExitStack

import numpy as np
import ml_dtypes

import concourse.bass as bass
import concourse.bacc as bacc
import concourse.tile as tile
from concourse import mybir
from concourse.bass_utils import run_bass_kernel_spmd

B, T, H = 8, 1024, 640
N_PHONEME, N_PHONE = 96, 230
N_CORES = 8
ROWS = B * T
RC = ROWS // N_CORES          # rows per core
NH = H // 128                 # contraction chunks
TW = 256                      # rows per enc DMA megatile
NB = RC // 128                # 128-row blocks per core
BF16 = ml_dtypes.bfloat16


def _structure(mapping: np.ndarray):
    """Segment-contiguous gather order, grouped by segment length (desc).

    Returns (col_ids, groups, perm):
      col_ids: phone index feeding each device matmul column (len NNZ)
      groups:  list of (L, nL, col_off, out_off) — nL segments of length L
               occupy matmul cols [col_off, col_off+nL*L) and device output
               cols [out_off, out_off+nL)
      perm:    perm[j] = original phoneme id of device output column j
    """
    segs = [np.nonzero(mapping[m] > 0)[0] for m in range(N_PHONEME)]
    lengths = np.array([len(s) for s in segs])
    assert lengths.min() >= 1
    order = np.argsort(-lengths, kind="stable")
    col_ids, groups, perm = [], [], []
    i = 0
    while i < N_PHONEME:
        L = int(lengths[order[i]])
        j = i
        while j < N_PHONEME and lengths[order[j]] == L:
            j += 1
        groups.append((L, j - i, len(col_ids), i))
        for k in range(i, j):
            m = int(order[k])
            col_ids.extend(segs[m].tolist())
            perm.append(m)
        i = j
    return np.array(col_ids, dtype=np.int64), groups, np.array(perm, dtype=np.int64)


def _patch_act_tables():
    """Make Exp and Ln resolve to the same activation-table set.

    bacc's insert_act_table_loads models a single table slot, so a kernel
    alternating Exp/Ln reloads a 1.3us table on every transition.  act_info
    has a joint set ('natural_log_exp_and_others') containing both; keep the
    set list's order/indices intact but strip Exp/Ln from the other sets so
    the pass picks the joint set for both and emits a single load.
    """
    if getattr(bacc, "_act_tables_patched", False):
        return
    from concourse import hw_specs
    orig = hw_specs.get_activation_tables
    act = mybir.ActivationFunctionType

    def patched(module_arch):
        tabs = orig(module_arch)
        joint = [k for k, v in tabs.items() if act.Exp in v and act.Ln in v]
        if not joint:
            return tabs
        j = joint[0]
        return {
            k: (v if k == j else (v - {act.Exp, act.Ln}))
            for k, v in tabs.items()
        }

    bacc.get_activation_tables = patched
    bacc._act_tables_patched = True


def _build_program(nnz: int, groups):
    """Build + compile the per-core Bass program. Returns the Bacc object."""
    _patch_act_tables()
    nc = bacc.Bacc("TRN2", target_bir_lowering=False, debug=False)
    dt = mybir.dt
    act = mybir.ActivationFunctionType

    # enc interleaved: [128, RC, NH]; element (p, r, c) = enc[r, c*128+p]
    enck_d = nc.dram_tensor("enck", [128, RC, NH], dt.bfloat16, kind="ExternalInput")
    # W interleaved: [128, NH, nnz]; element (p, c, n) = W[c*128+p, n]
    wk_d = nc.dram_tensor("wk", [128, NH, nnz], dt.bfloat16, kind="ExternalInput")
    # out packed: [128, NB, 96]; element (p, b, m) = out[b*128+p, m]
    out_d = nc.dram_tensor("out", [128, NB, N_PHONEME], dt.float32, kind="ExternalOutput")

    RT = TW // 128
    NMT = RC // TW

    with ExitStack() as ctx:
        tc = ctx.enter_context(tile.TileContext(nc))
        wpool = ctx.enter_context(tc.tile_pool(name="wpool", bufs=1))
        epool = ctx.enter_context(tc.tile_pool(name="epool", bufs=3))
        ppool = ctx.enter_context(tc.tile_pool(name="ppool", bufs=4, space="PSUM"))
        spool = ctx.enter_context(tc.tile_pool(name="spool", bufs=2))

        wt = wpool.tile([128, NH, nnz], dt.bfloat16)
        nc.sync.dma_start(wt[:], wk_d[:])

        # PE warmup: dummy matmuls on zeroed scratch run while the first
        # DMAs land, ramping the tensor engine's p-state so the real
        # stream starts at full speed.  They write the first megatile's
        # PSUM bank; the real accumulation (start=True) overwrites it.
        wu = wpool.tile([128, 512], dt.bfloat16)
        nc.vector.memset(wu[:], 0.0)
        ps0 = ppool.tile([128, RT, 512], dt.float32, tag="ps")
        for _ in range(8):
            nc.tensor.matmul(ps0[:, 0, :], wu[:, :128], wu[:], start=True, stop=True)

        for mt in range(NMT):
            et = epool.tile([128, TW, NH], dt.bfloat16)
            # issue enc loads from the Activation HWDGE so they don't
            # serialize behind the Sync engine's descriptor generation
            nc.scalar.dma_start(et[:], enck_d[:, mt * TW:(mt + 1) * TW, :])

            # one PSUM bank per block
            ps = ps0 if mt == 0 else ppool.tile([128, RT, 512], dt.float32, tag="ps")
            for r in range(RT):
                for c in range(NH):
                    nc.tensor.matmul(
                        ps[:, r, :nnz],
                        et[:, r * 128:(r + 1) * 128, c],
                        wt[:, c, :],
                        start=(c == 0),
                        stop=(c == NH - 1),
                    )

            # segment max: one strided reduce per length group, batched over
            # all RT row blocks (4D input AP [128, RT, nL, L])
            pmax = spool.tile([128, RT, N_PHONEME], dt.float32, tag="pmax")
            for (L, nL, coff, ooff) in groups:
                src = ps[:, :, coff:coff + nL * L].rearrange(
                    "p r (s l) -> p r s l", l=L
                )
                nc.vector.reduce_max(
                    pmax[:, :, ooff:ooff + nL], src, axis=mybir.AxisListType.X
                )

            # exp (no max-subtraction needed: |phone| <~ 7, exp fits fp32);
            # row-sum comes free via the activation accumulator
            ex = spool.tile([128, RT, N_PHONEME], dt.float32, tag="ex")
            se = spool.tile([128, RT], dt.float32, tag="se")
            for r in range(RT):
                nc.scalar.activation(ex[:, r, :], pmax[:, r, :], act.Exp,
                                     accum_out=se[:, r:r + 1])
            lse = spool.tile([128, RT], dt.float32, tag="lse")
            nc.scalar.activation(lse[:], se[:], act.Ln)
            ott = spool.tile([128, RT, N_PHONEME], dt.float32, tag="ott")
            for r in range(RT):
                nc.vector.tensor_scalar_sub(
                    ott[:, r, :], pmax[:, r, :], lse[:, r:r + 1]
                )
            nc.sync.dma_start(out_d[:, mt * RT:(mt + 1) * RT, :], ott[:])

    nc.compile()
    return nc


_CACHE: dict = {}


def _get_compiled(mapping: np.ndarray):
    key = mapping.astype(np.float32).tobytes()
    if _CACHE.get("key") != key:
        col_ids, groups, perm = _structure(mapping)
        nc = _build_program(len(col_ids), groups)
        _CACHE.update(key=key, col_ids=col_ids, groups=groups, perm=perm, nc=nc)
    return _CACHE["nc"], _CACHE["col_ids"], _CACHE["perm"]


def _prep_in_maps(enc_output, feature2phone, col_ids):
    scale = np.float32(1.0) / np.sqrt(np.float32(H))
    wg = (feature2phone.astype(np.float32) * scale)[:, col_ids].astype(BF16)
    # [H, nnz] -> [128, NH, nnz]
    wk = np.ascontiguousarray(wg.reshape(NH, 128, -1).transpose(1, 0, 2))
    # enc [ROWS, H] -> [128, ROWS, NH]
    e3 = enc_output.astype(BF16).reshape(ROWS, NH, 128)
    enck = np.ascontiguousarray(e3.transpose(2, 0, 1))
    in_maps = []
    for c in range(N_CORES):
        in_maps.append({
            "enck": np.ascontiguousarray(enck[:, c * RC:(c + 1) * RC, :]),
            "wk": wk,
        })
    return in_maps


def run_device(enc_output, feature2phone, mapping, trace=False, **kw):
    """Build/compile (cached), run on the 8 cores, return (output, BassKernelResults)."""
    enc_output = np.asarray(enc_output)
    feature2phone = np.asarray(feature2phone)
    mapping = np.asarray(mapping)
    nc, col_ids, perm = _get_compiled(mapping)
    in_maps = _prep_in_maps(enc_output, feature2phone, col_ids)
    res = run_bass_kernel_spmd(
        nc, in_maps, core_ids=list(range(N_CORES)), trace=trace, **kw
    )
    # device out [128, NB, 96] packed -> rows b*128+p
    dev = np.concatenate(
        [res.results[c]["out"].transpose(1, 0, 2).reshape(RC, N_PHONEME)
         for c in range(N_CORES)],
        axis=0,
    )
    out = np.empty_like(dev)
    out[:, perm] = dev
    return out.reshape(B, T, N_PHONEME).astype(np.float32), res


def kernel(enc_output, feature2phone, mapping):
    out, _ = run_device(enc_output, feature2phone, mapping)
    return out


# revision 20
# speedup vs baseline: 1.1715x; 1.0243x over previous
"""Trainium2 Bass kernel for CompositionalPhoneticsModel (segment_reduce).

Computation (reference):
    phone   = einsum('bth,hp->btp', enc_output, feature2phone) / sqrt(H)
    allo    = where(mapping>0, phone[:,:,None,:]*mapping, -inf)   # mapping is 0/1
    phoneme = max(allo, axis=-1)                                  # masked segment max
    out     = log_softmax(phoneme, axis=2)

Device strategy (8 NeuronCores, data-parallel over the B*T=8192 rows):
  * Host gathers feature2phone columns into segment-contiguous order
    (phones in 2 segments get duplicated columns; NNZ ~ 460), folds in the
    1/sqrt(H) scale, and sorts segments by length so the per-segment max is a
    handful of strided DVE reduce_max ops.  The device phoneme order is a
    permutation of 0..95; max/logsumexp are permutation-invariant so the host
    un-permutes the output columns at the end.
  * Host pre-casts enc to bf16 and pre-interleaves it as [128, rows, NH]
    (partition p holds h = c*128+p for all 5 contraction chunks c) so each
    DMA moves multi-KB contiguous lines per partition and the matmul lhsT
    tiles are direct slices — no on-chip transposes.
  * log-softmax without max-subtraction (phone logits are ~N(0,1); exp fits
    fp32 comfortably): exp on ScalarE (one table load), row-sums and the
    final x - ln(sum) on VectorE.
"""

from contextlib import ExitStack

import numpy as np
import ml_dtypes

import concourse.bass as bass
import concourse.bacc as bacc
import concourse.tile as tile
from concourse import mybir
from concourse.bass_utils import run_bass_kernel_spmd

B, T, H = 8, 1024, 640
N_PHONEME, N_PHONE = 96, 230
N_CORES = 8
ROWS = B * T
RC = ROWS // N_CORES          # rows per core
NH = H // 128                 # contraction chunks
TW = 256                      # rows per enc DMA megatile
NB = RC // 128                # 128-row blocks per core
BF16 = ml_dtypes.bfloat16


def _structure(mapping: np.ndarray):
    """Segment-contiguous gather order, grouped by segment length (desc).

    Returns (col_ids, groups, perm):
      col_ids: phone index feeding each device matmul column (len NNZ)
      groups:  list of (L, nL, col_off, out_off) — nL segments of length L
               occupy matmul cols [col_off, col_off+nL*L) and device output
               cols [out_off, out_off+nL)
      perm:    perm[j] = original phoneme id of device output column j
    """
    segs = [np.nonzero(mapping[m] > 0)[0] for m in range(N_PHONEME)]
    assert min(len(s) for s in segs) >= 1
    # pad segment lengths up to even targets (repeating a member doesn't
    # change the max): fewer distinct lengths -> fewer DVE reduce ops.
    # Only worthwhile while the matmul width stays within one PSUM bank.
    padded = []
    for s in segs:
        t = ((len(s) + 1) // 2) * 2
        padded.append(np.concatenate([s, np.full(t - len(s), s[0], s.dtype)]))
    if sum(len(s) for s in padded) <= 512:
        segs = padded
    lengths = np.array([len(s) for s in segs])
    order = np.argsort(-lengths, kind="stable")
    col_ids, groups, perm = [], [], []
    i = 0
    while i < N_PHONEME:
        L = int(lengths[order[i]])
        j = i
        while j < N_PHONEME and lengths[order[j]] == L:
            j += 1
        groups.append((L, j - i, len(col_ids), i))
        for k in range(i, j):
            m = int(order[k])
            col_ids.extend(segs[m].tolist())
            perm.append(m)
        i = j
    return np.array(col_ids, dtype=np.int64), groups, np.array(perm, dtype=np.int64)


def _patch_act_tables():
    """Make Exp and Ln resolve to the same activation-table set.

    bacc's insert_act_table_loads models a single table slot, so a kernel
    alternating Exp/Ln reloads a 1.3us table on every transition.  act_info
    has a joint set ('natural_log_exp_and_others') containing both; keep the
    set list's order/indices intact but strip Exp/Ln from the other sets so
    the pass picks the joint set for both and emits a single load.
    """
    if getattr(bacc, "_act_tables_patched", False):
        return
    from concourse import hw_specs
    orig = hw_specs.get_activation_tables
    act = mybir.ActivationFunctionType

    def patched(module_arch):
        tabs = orig(module_arch)
        joint = [k for k, v in tabs.items() if act.Exp in v and act.Ln in v]
        if not joint:
            return tabs
        j = joint[0]
        return {
            k: (v if k == j else (v - {act.Exp, act.Ln}))
            for k, v in tabs.items()
        }

    bacc.get_activation_tables = patched
    bacc._act_tables_patched = True


def _build_program(nnz: int, groups):
    """Build + compile the per-core Bass program. Returns the Bacc object."""
    _patch_act_tables()
    nc = bacc.Bacc("TRN2", target_bir_lowering=False, debug=False)
    dt = mybir.dt
    act = mybir.ActivationFunctionType

    # enc interleaved: [128, RC, NH]; element (p, r, c) = enc[r, c*128+p]
    enck_d = nc.dram_tensor("enck", [128, RC, NH], dt.bfloat16, kind="ExternalInput")
    # W interleaved: [128, NH, nnz]; element (p, c, n) = W[c*128+p, n]
    wk_d = nc.dram_tensor("wk", [128, NH, nnz], dt.bfloat16, kind="ExternalInput")
    # out packed: [128, NB, 96]; element (p, b, m) = out[b*128+p, m]
    out_d = nc.dram_tensor("out", [128, NB, N_PHONEME], dt.float32, kind="ExternalOutput")

    RT = TW // 128
    NMT = RC // TW

    with ExitStack() as ctx:
        tc = ctx.enter_context(tile.TileContext(nc))
        wpool = ctx.enter_context(tc.tile_pool(name="wpool", bufs=1))
        epool = ctx.enter_context(tc.tile_pool(name="epool", bufs=3))
        ppool = ctx.enter_context(tc.tile_pool(name="ppool", bufs=4, space="PSUM"))
        spool = ctx.enter_context(tc.tile_pool(name="spool", bufs=2))

        wt = wpool.tile([128, NH, nnz], dt.bfloat16)
        nc.sync.dma_start(wt[:], wk_d[:])

        # PE warmup: dummy matmuls on zeroed scratch run while the first
        # DMAs land, ramping the tensor engine's p-state so the real
        # stream starts at full speed.  They write the first megatile's
        # PSUM bank; the real accumulation (start=True) overwrites it.
        wu = wpool.tile([128, 512], dt.bfloat16)
        nc.vector.memset(wu[:], 0.0)
        ps0 = ppool.tile([128, RT, 512], dt.float32, tag="ps")
        for _ in range(8):
            nc.tensor.matmul(ps0[:, 0, :], wu[:, :128], wu[:], start=True, stop=True)

        for mt in range(NMT):
            et = epool.tile([128, TW, NH], dt.bfloat16)
            # issue enc loads from the Activation HWDGE so they don't
            # serialize behind the Sync engine's descriptor generation
            nc.scalar.dma_start(et[:], enck_d[:, mt * TW:(mt + 1) * TW, :])

            # one PSUM bank per block
            ps = ps0 if mt == 0 else ppool.tile([128, RT, 512], dt.float32, tag="ps")
            for r in range(RT):
                for c in range(NH):
                    nc.tensor.matmul(
                        ps[:, r, :nnz],
                        et[:, r * 128:(r + 1) * 128, c],
                        wt[:, c, :],
                        start=(c == 0),
                        stop=(c == NH - 1),
                    )

            # segment max: one strided reduce per length group, batched over
            # all RT row blocks (4D input AP [128, RT, nL, L])
            pmax = spool.tile([128, RT, N_PHONEME], dt.float32, tag="pmax")
            for (L, nL, coff, ooff) in groups:
                src = ps[:, :, coff:coff + nL * L].rearrange(
                    "p r (s l) -> p r s l", l=L
                )
                nc.vector.reduce_max(
                    pmax[:, :, ooff:ooff + nL], src, axis=mybir.AxisListType.X
                )

            # exp (no max-subtraction needed: |phone| <~ 7, exp fits fp32);
            # row-sum comes free via the activation accumulator
            ex = spool.tile([128, RT, N_PHONEME], dt.float32, tag="ex")
            se = spool.tile([128, RT], dt.float32, tag="se")
            for r in range(RT):
                nc.scalar.activation(ex[:, r, :], pmax[:, r, :], act.Exp,
                                     accum_out=se[:, r:r + 1])
            lse = spool.tile([128, RT], dt.float32, tag="lse")
            nc.scalar.activation(lse[:], se[:], act.Ln)
            ott = spool.tile([128, RT, N_PHONEME], dt.float32, tag="ott")
            for r in range(RT):
                nc.vector.tensor_scalar_sub(
                    ott[:, r, :], pmax[:, r, :], lse[:, r:r + 1]
                )
            nc.sync.dma_start(out_d[:, mt * RT:(mt + 1) * RT, :], ott[:])

    nc.compile()
    return nc


_CACHE: dict = {}


def _get_compiled(mapping: np.ndarray):
    key = mapping.astype(np.float32).tobytes()
    if _CACHE.get("key") != key:
        col_ids, groups, perm = _structure(mapping)
        nc = _build_program(len(col_ids), groups)
        _CACHE.update(key=key, col_ids=col_ids, groups=groups, perm=perm, nc=nc)
    return _CACHE["nc"], _CACHE["col_ids"], _CACHE["perm"]


def _prep_in_maps(enc_output, feature2phone, col_ids):
    scale = np.float32(1.0) / np.sqrt(np.float32(H))
    wg = (feature2phone.astype(np.float32) * scale)[:, col_ids].astype(BF16)
    # [H, nnz] -> [128, NH, nnz]
    wk = np.ascontiguousarray(wg.reshape(NH, 128, -1).transpose(1, 0, 2))
    # enc [ROWS, H] -> [128, ROWS, NH]
    e3 = enc_output.astype(BF16).reshape(ROWS, NH, 128)
    enck = np.ascontiguousarray(e3.transpose(2, 0, 1))
    in_maps = []
    for c in range(N_CORES):
        in_maps.append({
            "enck": np.ascontiguousarray(enck[:, c * RC:(c + 1) * RC, :]),
            "wk": wk,
        })
    return in_maps


def run_device(enc_output, feature2phone, mapping, trace=False, **kw):
    """Build/compile (cached), run on the 8 cores, return (output, BassKernelResults)."""
    enc_output = np.asarray(enc_output)
    feature2phone = np.asarray(feature2phone)
    mapping = np.asarray(mapping)
    nc, col_ids, perm = _get_compiled(mapping)
    in_maps = _prep_in_maps(enc_output, feature2phone, col_ids)
    res = run_bass_kernel_spmd(
        nc, in_maps, core_ids=list(range(N_CORES)), trace=trace, **kw
    )
    # device out [128, NB, 96] packed -> rows b*128+p
    dev = np.concatenate(
        [res.results[c]["out"].transpose(1, 0, 2).reshape(RC, N_PHONEME)
         for c in range(N_CORES)],
        axis=0,
    )
    out = np.empty_like(dev)
    out[:, perm] = dev
    return out.reshape(B, T, N_PHONEME).astype(np.float32), res


def kernel(enc_output, feature2phone, mapping):
    out, _ = run_device(enc_output, feature2phone, mapping)
    return out
